# revision 52
# baseline (speedup 1.0000x reference)
"""Trainium2 Bass kernel for nn_CANDY_41077067219071.

Computation (per channel c of 64, H = I = 1024):
    S     = x[c] * clamp(p_mask)                         # elementwise
    t     = Wp_eff @ S            ; u  = clamp(t)        # MM1
    v     = clamp(u @ p_lin_w.T + p_b)                   # MM2  (p_out)
    z     = Wzp @ v               ; w  = clamp(z)        # MM3
    y     = clamp(w @ z_lin_w.T + z_b)                   # MM4  (z_out)
    out[c] = v + y
Sharding: channels split 8 per NeuronCore (pure data parallel), weights
replicated.  The chain alternates between natural and transposed layouts
so every intermediate is directly usable as the next matmul's stationary
(lhsT) operand -- no transposes anywhere:

    MM1: lhsT=S[k,i]   rhs=Wp_eff.T[k,h]  -> tT[i,h]
    MM2: lhsT=uT[i,h]  rhs=p_lin_w.T[i,j] -> v[h,j]
    MM3: lhsT=v[h,j]   rhs=Wzp.T[h,g]     -> zT[j,g]
    MM4: lhsT=wT[j,g]  rhs=z_lin_w.T[j,m] -> y[g,m]

All four matmuls run with float16 operands (fp32 PSUM accumulate):
 - fp16 streams at the PE's full 1 column/cycle (f32r moving pays ~1.125
   cycles and fp32 4x), and fp16 stationaries get the fast-weight-load
   path, so LDWEIGHTS hides under the previous matmul's stream;
 - emitting k innermost with all free-dim tiles of one stationary
   adjacent keeps the weight reload off the critical row pipe: measured
   cadence is ~213ns per N=512 matmul (=512 cycles, the stream wall) vs
   ~280ns for the naive ordering;
 - MM1 exploits the upper-triangular rhs at N=256 granularity (56% of
   the dense stream instead of 75% at N=512); each [P,256] quarter gets
   its own PSUM bank -- matmul PSUM outputs must be bank-aligned, and
   interleaved accumulation groups in one bank corrupt results;
 - fp16 weights/inputs halve SBUF and HBM traffic, so all four weight
   matrices stay resident (no per-channel re-streaming);
 - measured end-to-end rel err 9.1e-3 vs the 2e-2 budget (fp32 chain
   would be ~1e-3: the fp16 quantization of MM1's operands dominates).
"""

import os
import sys

for _p in ("/root/.axon_site/_ro/trn_rl_repo", "/opt/trn_rl_repo"):
    if os.path.isdir(_p) and _p not in sys.path:
        sys.path.append(_p)

import numpy as np

import concourse.bass as bass
import concourse.mybir as mybir
from concourse import bacc
from concourse.tile import TileContext
from concourse.bass_utils import run_bass_kernel_spmd

H = 1024          # hidden == input size
C = 64            # channels
NCORES = 8
CLOC = C // NCORES  # channels per core
P = 128           # SBUF partitions
KO = H // P       # 8 k-blocks
NT = 512          # matmul free-dim tile (1 fp32 PSUM bank)
NN = H // NT      # 2 free-dim tiles

f32 = mybir.dt.float32
f32r = mybir.dt.float32r
f16 = mybir.dt.float16

_cache = {}

# Set by kernel() after each run (for test harness inspection).
last_results = None


def _build(has_pb: bool, has_zb: bool) -> bass.Bass:
    nc = bacc.Bacc(debug=False)

    x = nc.declare_dram_parameter("x", [CLOC, H, H], f16, isOutput=False)
    mask = nc.declare_dram_parameter("mask", [H, H], f16, isOutput=False)
    w0 = nc.declare_dram_parameter("w0", [H, H], f16, isOutput=False)
    wf = [
        nc.declare_dram_parameter(f"w{i}", [H, H], f16, isOutput=False)
        for i in (1, 2, 3)
    ]
    pb = zb = None
    if has_pb:
        pb = nc.declare_dram_parameter("pb", [1, H], f16, isOutput=False)
    if has_zb:
        zb = nc.declare_dram_parameter("zb", [1, H], f16, isOutput=False)
    # fp16 output: halves the out DMA; host upcasts to f32 (v+y is in
    # [-2,2], so the fp16 write adds ~5e-4 abs err vs the 2e-2 budget)
    out = nc.declare_dram_parameter("out", [CLOC, H, H], f16, isOutput=True)

    xr = x.ap().rearrange("c (ko p) i -> c p ko i", p=P)
    maskr = mask.ap().rearrange("(ko p) i -> p ko i", p=P)
    w0r = w0.ap().rearrange("(ko p) n -> p ko n", p=P)
    wfr = [w.ap().rearrange("(ko p) n -> p ko n", p=P) for w in wf]
    outr = out.ap().rearrange("c (go p) m -> c p go m", p=P)

    with TileContext(nc) as tc:
        with (
            tc.tile_pool(name="const", bufs=1) as constp,
            tc.tile_pool(name="spool", bufs=1) as spool,
            tc.tile_pool(name="uwpool", bufs=1) as uwpool,
            tc.tile_pool(name="vpool", bufs=1) as vpool,
            tc.tile_pool(name="ytp", bufs=3) as ytp,
            tc.tile_pool(name="outp", bufs=2) as outp,
            tc.tile_pool(name="psum", bufs=8, space="PSUM") as psum,
        ):
            # ---- resident tensors, loaded once ----
            mask_sb = constp.tile([P, KO, H], f16, tag="mask")
            w0_sb = constp.tile([P, KO, H], f16, tag="w0")
            wf_sb = [
                constp.tile([P, KO, H], f16, tag=f"w{i}", name=f"w{i}_sb")
                for i in (1, 2, 3)
            ]

            # PE p-state warmup: dummy matmuls from t~1.5us so the
            # 1.2->2.4GHz DVFS ramp (~3us of continuous execution)
            # completes during the DMA lead-in; channel 0's MM1 is
            # compute-bound at ramp speed without this.
            warm_w = constp.tile([P, P], f16, tag="warmw")
            warm_m = constp.tile([P, NT], f16, tag="warmm")
            nc.vector.memset(warm_w[:], 0.0)
            nc.vector.memset(warm_m[:], 0.0)
            warm_ps = psum.tile([P, NT], f32, tag="ps", name="warm_ps")
            for _ in range(20):
                nc.tensor.matmul(
                    warm_ps[:], warm_w[:], warm_m[:], start=True, stop=True
                )

            # mask + w0 interleaved k-chunks on scalar (the first matmul
            # needs mask0/w0_0); w0 upper-tri: cols 0:NT of k-blocks 4..7
            # are never read (tri-skip) -- don't transfer
            for ko in range(KO):
                nc.scalar.dma_start(mask_sb[:, ko, :], maskr[:, ko, :])
                if ko < 4:
                    nc.scalar.dma_start(w0_sb[:, ko, :], w0r[:, ko, :])
                else:
                    nc.scalar.dma_start(
                        w0_sb[:, ko, NT:], w0r[:, ko, NT:]
                    )
            for i in range(3):
                nc.scalar.dma_start(wf_sb[i][:, :, :], wfr[i][:, :, :])

            ones_sb = None
            pb_sb = zb_sb = None
            if has_pb or has_zb:
                ones_sb = constp.tile([1, P], f16, tag="ones")
                nc.vector.memset(ones_sb[:], 1.0)
            if has_pb:
                pb_sb = constp.tile([1, H], f16, tag="pb")
                nc.sync.dma_start(pb_sb[:], pb.ap())
            if has_zb:
                zb_sb = constp.tile([1, H], f16, tag="zb")
                nc.sync.dma_start(zb_sb[:], zb.ap())

            def load_s(c):
                # x is fp16 in DRAM: land it straight in the S tile, then
                # multiply by the mask in place.  Channel 0's muls run on
                # the 3x-faster, startup-idle DVE (its chunk readiness
                # gates the pipeline fill); later channels use gpsimd so
                # the DVE FIFO stays clear for PSUM-drain clamps.
                s = spool.tile([P, KO, H], f16, tag="S")
                for ko in range(KO):
                    nc.sync.dma_start(s[:, ko, :], xr[c, :, ko, :])
                    eng = nc.vector if c == 0 else nc.gpsimd
                    eng.tensor_mul(
                        s[:, ko, :], s[:, ko, :], mask_sb[:, ko, :]
                    )
                return s

            def mm_layer(lhsT_sb, rhs_sb, bias_sb, writer):
                # out[m*P:(m+1)*P, :] = lhsT.T @ rhs (+bias).  k innermost
                # with both nt halves adjacent: consecutive matmuls share
                # the same stationary lhsT[:, k, m-block], so the PE can
                # skip / overlap the redundant weight reload.
                for m in range(KO):
                    ps0 = psum.tile([P, NT], f32, tag="ps")
                    ps1 = psum.tile([P, NT], f32, tag="ps")
                    pss = (ps0, ps1)
                    for k in range(KO):
                        for nt in range(NN):
                            nc.tensor.matmul(
                                pss[nt][:],
                                lhsT_sb[:, k, m * P:(m + 1) * P],
                                rhs_sb[:, k, nt * NT:(nt + 1) * NT],
                                start=(k == 0),
                                stop=(k == KO - 1 and bias_sb is None),
                            )
                    if bias_sb is not None:
                        # rank-1 accumulate: ones[1,P].T @ bias[1,NT]
                        for nt in range(NN):
                            nc.tensor.matmul(
                                pss[nt][:],
                                ones_sb[:, :],
                                bias_sb[:, nt * NT:(nt + 1) * NT],
                                start=False,
                                stop=True,
                            )
                    for nt in range(NN):
                        writer(m, nt, pss[nt][:])

            NT1 = 256                  # MM1 free-dim tile: finer tri-skip
            NN1 = H // NT1             # 4 col tiles
            # col tile j only needs k-blocks with k*P < (j+1)*NT1
            khi1 = [min(KO, 2 * j + 2) for j in range(NN1)]

            def mm1_layer(lhsT_sb, rhs_sb, writer):
                # Upper-triangular rhs at NT1 granularity: col tile j only
                # needs k-blocks up to khi1[j].  Each [P, NT1] quarter gets
                # its own PSUM bank (matmul PSUM outputs must be bank-
                # aligned; the tail half of the bank is unused).  k is
                # outermost so consecutive matmuls share the stationary
                # lhsT[:, k, m-block] and the reload stays hidden.
                for m in range(KO):
                    quarter = []
                    for j in range(NN1):
                        q = psum.tile([P, NT], f32, tag="ps", name="q")
                        quarter.append(q[:, 0:NT1])
                    for k in range(KO):
                        for j in range(NN1):
                            if k >= khi1[j]:
                                continue
                            nc.tensor.matmul(
                                quarter[j],
                                lhsT_sb[:, k, m * P:(m + 1) * P],
                                rhs_sb[:, k, j * NT1:(j + 1) * NT1],
                                start=(k == 0),
                                stop=(k == khi1[j] - 1),
                            )
                            if k == khi1[j] - 1:
                                writer(m, j, quarter[j])

            def clamp_into(dst_sb, w=NT):
                def _w(m, nt, ps):
                    nc.vector.tensor_scalar(
                        dst_sb[:, m, nt * w:(nt + 1) * w],
                        ps,
                        1.0,
                        -1.0,
                        mybir.AluOpType.min,
                        mybir.AluOpType.max,
                    )
                return _w

            def mm1_fill(lhsT_sb, rhs_sb, writer):
                # Channel-0 MM1 only: the pipeline fill is gated by x/mask
                # k-chunk arrival (~1.6us apart), so order the work to
                # minimize what depends on late chunks.  Left-half
                # quarters (j=0,1; k<=3) of every m run while chunks land;
                # the right half goes k-outer in two m-groups of 4 (8 PSUM
                # banks each), leaving only the m-group tails dependent on
                # the last chunks.
                def q_tile():
                    t = psum.tile([P, NT], f32, tag="ps", name="q")
                    return t[:, 0:NT1]

                for m in range(KO):
                    qs = {j: q_tile() for j in (0, 1)}
                    for k in range(4):
                        for j in (0, 1):
                            if k < khi1[j]:
                                nc.tensor.matmul(
                                    qs[j],
                                    lhsT_sb[:, k, m * P:(m + 1) * P],
                                    rhs_sb[:, k, j * NT1:(j + 1) * NT1],
                                    start=(k == 0),
                                    stop=(k == khi1[j] - 1),
                                )
                                if k == khi1[j] - 1:
                                    writer(m, j, qs[j])
                for mg in range(2):
                    ms = range(mg * 4, mg * 4 + 4)
                    qs = {(m, j): q_tile() for m in ms for j in (2, 3)}
                    for k in range(KO):
                        for m in ms:
                            for j in (2, 3):
                                if k < khi1[j]:
                                    nc.tensor.matmul(
                                        qs[(m, j)],
                                        lhsT_sb[:, k, m * P:(m + 1) * P],
                                        rhs_sb[:, k, j * NT1:(j + 1) * NT1],
                                        start=(k == 0),
                                        stop=(k == khi1[j] - 1),
                                    )
                                    if k == khi1[j] - 1:
                                        writer(m, j, qs[(m, j)])

            s_cur = load_s(0)

            for c in range(CLOC):
                uw = uwpool.tile([P, KO, H], f16, tag="uw")   # uT
                v = vpool.tile([P, KO, H], f16, tag="v")

                mm1 = mm1_fill if c == 0 else mm1_layer
                mm1(s_cur, w0_sb, clamp_into(uw, NT1))

                # next channel's x can start loading as soon as MM1 is done
                # with S (spool bufs=1 enforces that)
                if c + 1 < CLOC:
                    s_next = load_s(c + 1)

                mm_layer(uw, wf_sb[0], pb_sb, clamp_into(v))

                wt2 = uwpool.tile([P, KO, H], f16, tag="uw")  # wT reuses slot
                mm_layer(v, wf_sb[1], None, clamp_into(wt2))

                orow = [None]

                def final_writer(m, nt, ps, c=c, v=v, orow=orow):
                    yt = ytp.tile([P, NT], f16, tag="yt")
                    nc.vector.tensor_scalar(
                        yt[:],
                        ps,
                        1.0,
                        -1.0,
                        mybir.AluOpType.min,
                        mybir.AluOpType.max,
                    )
                    if nt == 0:
                        orow[0] = outp.tile([P, H], f16, tag="out", name="ot")
                    ot = orow[0]
                    nc.vector.tensor_add(
                        ot[:, nt * NT:(nt + 1) * NT],
                        yt[:],
                        v[:, m, nt * NT:(nt + 1) * NT],
                    )
                    if nt == NN - 1:
                        # one whole-row DMA per m-block on the (idle after
                        # startup) scalar queue: fewer, larger transfers
                        nc.scalar.dma_start(outr[c, :, m, :], ot[:, :])

                mm_layer(wt2, wf_sb[2], zb_sb, final_writer)

                if c + 1 < CLOC:
                    s_cur = s_next

    nc.compile()  # bacc passes: split multi-waits into event semaphores etc.
    return nc


def _prep_host(x, p_mask, Wp, Wp_diag, Wzp, p_lin_w, p_lin_b, z_lin_w,
               z_lin_b):
    x = np.ascontiguousarray(
        np.asarray(x, dtype=np.float32).reshape(C, H, H).astype(np.float16)
    )
    mask = np.clip(np.asarray(p_mask, dtype=np.float32), -1.0, 1.0)
    mask = np.ascontiguousarray(mask.astype(np.float16))

    Wp = np.asarray(Wp, dtype=np.float32)
    Wp_eff = np.tril(Wp)
    idx = np.arange(H)
    Wp_eff[idx, idx] = np.clip(np.diagonal(Wp), 0.0, 1.0) + np.asarray(
        Wp_diag, dtype=np.float32
    )
    w0 = np.ascontiguousarray(Wp_eff.T.astype(np.float16))
    wf = [
        np.ascontiguousarray(np.asarray(p_lin_w, dtype=np.float32).T.astype(np.float16)),
        np.ascontiguousarray(np.asarray(Wzp, dtype=np.float32).T.astype(np.float16)),
        np.ascontiguousarray(np.asarray(z_lin_w, dtype=np.float32).T.astype(np.float16)),
    ]
    pb = np.asarray(p_lin_b, dtype=np.float32).reshape(1, H).astype(np.float16)
    zb = np.asarray(z_lin_b, dtype=np.float32).reshape(1, H).astype(np.float16)
    return x, mask, w0, wf, pb, zb


def kernel(x, p_mask, Wp, Wp_diag, Wzp, p_lin_w, p_lin_b, z_lin_w, z_lin_b):
    global last_results
    x, mask, w0, wf, pb, zb = _prep_host(
        x, p_mask, Wp, Wp_diag, Wzp, p_lin_w, p_lin_b, z_lin_w, z_lin_b
    )
    has_pb = bool(np.any(pb))
    has_zb = bool(np.any(zb))

    key = (has_pb, has_zb)
    if key not in _cache:
        _cache[key] = _build(has_pb, has_zb)
    nc = _cache[key]

    in_maps = []
    for core in range(NCORES):
        m = {
            "x": x[core * CLOC:(core + 1) * CLOC],
            "mask": mask,
            "w0": w0,
            "w1": wf[0],
            "w2": wf[1],
            "w3": wf[2],
        }
        if has_pb:
            m["pb"] = pb
        if has_zb:
            m["zb"] = zb
        in_maps.append(m)

    want_trace = bool(os.environ.get("BASS_TRACE"))
    try:
        res = run_bass_kernel_spmd(
            nc, in_maps, list(range(NCORES)), trace=want_trace
        )
    except ModuleNotFoundError:
        if not want_trace:
            raise
        # profiling hook unavailable in this environment -- run untraced
        res = run_bass_kernel_spmd(
            nc, in_maps, list(range(NCORES)), trace=False
        )
    last_results = res
    out = np.concatenate([r["out"] for r in res.results], axis=0)
    return out.reshape(1, C, H, H).astype(np.float32)


# revision 53
# speedup vs baseline: 1.0034x; 1.0034x over previous
"""Trainium2 Bass kernel for nn_CANDY_41077067219071.

Computation (per channel c of 64, H = I = 1024):
    S     = x[c] * clamp(p_mask)                         # elementwise
    t     = Wp_eff @ S            ; u  = clamp(t)        # MM1
    v     = clamp(u @ p_lin_w.T + p_b)                   # MM2  (p_out)
    z     = Wzp @ v               ; w  = clamp(z)        # MM3
    y     = clamp(w @ z_lin_w.T + z_b)                   # MM4  (z_out)
    out[c] = v + y
Sharding: channels split 8 per NeuronCore (pure data parallel), weights
replicated.  The chain alternates between natural and transposed layouts
so every intermediate is directly usable as the next matmul's stationary
(lhsT) operand -- no transposes anywhere:

    MM1: lhsT=S[k,i]   rhs=Wp_eff.T[k,h]  -> tT[i,h]
    MM2: lhsT=uT[i,h]  rhs=p_lin_w.T[i,j] -> v[h,j]
    MM3: lhsT=v[h,j]   rhs=Wzp.T[h,g]     -> zT[j,g]
    MM4: lhsT=wT[j,g]  rhs=z_lin_w.T[j,m] -> y[g,m]

All four matmuls run with float16 operands (fp32 PSUM accumulate):
 - fp16 streams at the PE's full 1 column/cycle (f32r moving pays ~1.125
   cycles and fp32 4x), and fp16 stationaries get the fast-weight-load
   path, so LDWEIGHTS hides under the previous matmul's stream;
 - emitting k innermost with all free-dim tiles of one stationary
   adjacent keeps the weight reload off the critical row pipe: measured
   cadence is ~213ns per N=512 matmul (=512 cycles, the stream wall) vs
   ~280ns for the naive ordering;
 - MM1 exploits the upper-triangular rhs at N=256 granularity (56% of
   the dense stream instead of 75% at N=512); each [P,256] quarter gets
   its own PSUM bank -- matmul PSUM outputs must be bank-aligned, and
   interleaved accumulation groups in one bank corrupt results;
 - fp16 weights/inputs halve SBUF and HBM traffic, so all four weight
   matrices stay resident (no per-channel re-streaming);
 - measured end-to-end rel err 9.1e-3 vs the 2e-2 budget (fp32 chain
   would be ~1e-3: the fp16 quantization of MM1's operands dominates).
"""

import os
import sys

for _p in ("/root/.axon_site/_ro/trn_rl_repo", "/opt/trn_rl_repo"):
    if os.path.isdir(_p) and _p not in sys.path:
        sys.path.append(_p)

import numpy as np

import concourse.bass as bass
import concourse.mybir as mybir
from concourse import bacc
from concourse.tile import TileContext
from concourse.bass_utils import run_bass_kernel_spmd

H = 1024          # hidden == input size
C = 64            # channels
NCORES = 8
CLOC = C // NCORES  # channels per core
P = 128           # SBUF partitions
KO = H // P       # 8 k-blocks
NT = 512          # matmul free-dim tile (1 fp32 PSUM bank)
NN = H // NT      # 2 free-dim tiles

f32 = mybir.dt.float32
f32r = mybir.dt.float32r
f16 = mybir.dt.float16

_cache = {}

# Set by kernel() after each run (for test harness inspection).
last_results = None


def _build(has_pb: bool, has_zb: bool) -> bass.Bass:
    nc = bacc.Bacc(debug=False)

    x = nc.declare_dram_parameter("x", [CLOC, H, H], f16, isOutput=False)
    mask = nc.declare_dram_parameter("mask", [H, H], f16, isOutput=False)
    w0 = nc.declare_dram_parameter("w0", [H, H], f16, isOutput=False)
    wf = [
        nc.declare_dram_parameter(f"w{i}", [H, H], f16, isOutput=False)
        for i in (1, 2, 3)
    ]
    pb = zb = None
    if has_pb:
        pb = nc.declare_dram_parameter("pb", [1, H], f16, isOutput=False)
    if has_zb:
        zb = nc.declare_dram_parameter("zb", [1, H], f16, isOutput=False)
    # fp16 output: halves the out DMA; host upcasts to f32 (v+y is in
    # [-2,2], so the fp16 write adds ~5e-4 abs err vs the 2e-2 budget)
    out = nc.declare_dram_parameter("out", [CLOC, H, H], f16, isOutput=True)

    xr = x.ap().rearrange("c (ko p) i -> c p ko i", p=P)
    maskr = mask.ap().rearrange("(ko p) i -> p ko i", p=P)
    w0r = w0.ap().rearrange("(ko p) n -> p ko n", p=P)
    wfr = [w.ap().rearrange("(ko p) n -> p ko n", p=P) for w in wf]
    outr = out.ap().rearrange("c (go p) m -> c p go m", p=P)

    with TileContext(nc) as tc:
        with (
            tc.tile_pool(name="const", bufs=1) as constp,
            tc.tile_pool(name="spool", bufs=1) as spool,
            tc.tile_pool(name="uwpool", bufs=1) as uwpool,
            tc.tile_pool(name="vpool", bufs=1) as vpool,
            tc.tile_pool(name="ytp", bufs=3) as ytp,
            tc.tile_pool(name="outp", bufs=2) as outp,
            tc.tile_pool(name="psum", bufs=8, space="PSUM") as psum,
        ):
            # ---- resident tensors, loaded once ----
            mask_sb = constp.tile([P, KO, H], f16, tag="mask")
            w0_sb = constp.tile([P, KO, H], f16, tag="w0")
            wf_sb = [
                constp.tile([P, KO, H], f16, tag=f"w{i}", name=f"w{i}_sb")
                for i in (1, 2, 3)
            ]

            # mask + w0 interleaved k-chunks on scalar (the first matmul
            # needs mask0/w0_0); w0 upper-tri: cols 0:NT of k-blocks 4..7
            # are never read (tri-skip) -- don't transfer
            for ko in range(KO):
                nc.scalar.dma_start(mask_sb[:, ko, :], maskr[:, ko, :])
                if ko < 4:
                    nc.scalar.dma_start(w0_sb[:, ko, :], w0r[:, ko, :])
                else:
                    nc.scalar.dma_start(
                        w0_sb[:, ko, NT:], w0r[:, ko, NT:]
                    )
            for i in range(3):
                nc.scalar.dma_start(wf_sb[i][:, :, :], wfr[i][:, :, :])

            ones_sb = None
            pb_sb = zb_sb = None
            if has_pb or has_zb:
                ones_sb = constp.tile([1, P], f16, tag="ones")
                nc.vector.memset(ones_sb[:], 1.0)
            if has_pb:
                pb_sb = constp.tile([1, H], f16, tag="pb")
                nc.sync.dma_start(pb_sb[:], pb.ap())
            if has_zb:
                zb_sb = constp.tile([1, H], f16, tag="zb")
                nc.sync.dma_start(zb_sb[:], zb.ap())

            def load_s(c):
                # x is fp16 in DRAM: land it straight in the S tile, then
                # multiply by the mask in place.  Channel 0's muls run on
                # the 3x-faster, startup-idle DVE (its chunk readiness
                # gates the pipeline fill); later channels use gpsimd so
                # the DVE FIFO stays clear for PSUM-drain clamps.
                s = spool.tile([P, KO, H], f16, tag="S")
                for ko in range(KO):
                    nc.sync.dma_start(s[:, ko, :], xr[c, :, ko, :])
                    eng = nc.vector if c == 0 else nc.gpsimd
                    eng.tensor_mul(
                        s[:, ko, :], s[:, ko, :], mask_sb[:, ko, :]
                    )
                return s

            def mm_layer(lhsT_sb, rhs_sb, bias_sb, writer):
                # out[m*P:(m+1)*P, :] = lhsT.T @ rhs (+bias).  k innermost
                # with both nt halves adjacent: consecutive matmuls share
                # the same stationary lhsT[:, k, m-block], so the PE can
                # skip / overlap the redundant weight reload.
                for m in range(KO):
                    ps0 = psum.tile([P, NT], f32, tag="ps")
                    ps1 = psum.tile([P, NT], f32, tag="ps")
                    pss = (ps0, ps1)
                    for k in range(KO):
                        for nt in range(NN):
                            nc.tensor.matmul(
                                pss[nt][:],
                                lhsT_sb[:, k, m * P:(m + 1) * P],
                                rhs_sb[:, k, nt * NT:(nt + 1) * NT],
                                start=(k == 0),
                                stop=(k == KO - 1 and bias_sb is None),
                            )
                    if bias_sb is not None:
                        # rank-1 accumulate: ones[1,P].T @ bias[1,NT]
                        for nt in range(NN):
                            nc.tensor.matmul(
                                pss[nt][:],
                                ones_sb[:, :],
                                bias_sb[:, nt * NT:(nt + 1) * NT],
                                start=False,
                                stop=True,
                            )
                    for nt in range(NN):
                        writer(m, nt, pss[nt][:])

            NT1 = 256                  # MM1 free-dim tile: finer tri-skip
            NN1 = H // NT1             # 4 col tiles
            # col tile j only needs k-blocks with k*P < (j+1)*NT1
            khi1 = [min(KO, 2 * j + 2) for j in range(NN1)]

            def mm1_layer(lhsT_sb, rhs_sb, writer):
                # Upper-triangular rhs at NT1 granularity: col tile j only
                # needs k-blocks up to khi1[j].  Each [P, NT1] quarter gets
                # its own PSUM bank (matmul PSUM outputs must be bank-
                # aligned; the tail half of the bank is unused).  k is
                # outermost so consecutive matmuls share the stationary
                # lhsT[:, k, m-block] and the reload stays hidden.
                for m in range(KO):
                    quarter = []
                    for j in range(NN1):
                        q = psum.tile([P, NT], f32, tag="ps", name="q")
                        quarter.append(q[:, 0:NT1])
                    for k in range(KO):
                        for j in range(NN1):
                            if k >= khi1[j]:
                                continue
                            nc.tensor.matmul(
                                quarter[j],
                                lhsT_sb[:, k, m * P:(m + 1) * P],
                                rhs_sb[:, k, j * NT1:(j + 1) * NT1],
                                start=(k == 0),
                                stop=(k == khi1[j] - 1),
                            )
                            if k == khi1[j] - 1:
                                writer(m, j, quarter[j])

            def clamp_into(dst_sb, w=NT):
                def _w(m, nt, ps):
                    nc.vector.tensor_scalar(
                        dst_sb[:, m, nt * w:(nt + 1) * w],
                        ps,
                        1.0,
                        -1.0,
                        mybir.AluOpType.min,
                        mybir.AluOpType.max,
                    )
                return _w

            def mm1_fill(lhsT_sb, rhs_sb, writer):
                # Channel-0 MM1 only: the pipeline fill is gated by x/mask
                # k-chunk arrival (~1.6us apart), so order the work to
                # minimize what depends on late chunks.  Left-half
                # quarters (j=0,1; k<=3) of every m run while chunks land;
                # the right half goes k-outer in two m-groups of 4 (8 PSUM
                # banks each), leaving only the m-group tails dependent on
                # the last chunks.
                def q_tile():
                    t = psum.tile([P, NT], f32, tag="ps", name="q")
                    return t[:, 0:NT1]

                for m in range(KO):
                    qs = {j: q_tile() for j in (0, 1)}
                    for k in range(4):
                        for j in (0, 1):
                            if k < khi1[j]:
                                nc.tensor.matmul(
                                    qs[j],
                                    lhsT_sb[:, k, m * P:(m + 1) * P],
                                    rhs_sb[:, k, j * NT1:(j + 1) * NT1],
                                    start=(k == 0),
                                    stop=(k == khi1[j] - 1),
                                )
                                if k == khi1[j] - 1:
                                    writer(m, j, qs[j])
                for mg in range(2):
                    ms = range(mg * 4, mg * 4 + 4)
                    qs = {(m, j): q_tile() for m in ms for j in (2, 3)}
                    for k in range(KO):
                        for m in ms:
                            for j in (2, 3):
                                if k < khi1[j]:
                                    nc.tensor.matmul(
                                        qs[(m, j)],
                                        lhsT_sb[:, k, m * P:(m + 1) * P],
                                        rhs_sb[:, k, j * NT1:(j + 1) * NT1],
                                        start=(k == 0),
                                        stop=(k == khi1[j] - 1),
                                    )
                                    if k == khi1[j] - 1:
                                        writer(m, j, qs[(m, j)])

            s_cur = load_s(0)

            for c in range(CLOC):
                uw = uwpool.tile([P, KO, H], f16, tag="uw")   # uT
                v = vpool.tile([P, KO, H], f16, tag="v")

                mm1 = mm1_fill if c == 0 else mm1_layer
                mm1(s_cur, w0_sb, clamp_into(uw, NT1))

                # next channel's x can start loading as soon as MM1 is done
                # with S (spool bufs=1 enforces that)
                if c + 1 < CLOC:
                    s_next = load_s(c + 1)

                mm_layer(uw, wf_sb[0], pb_sb, clamp_into(v))

                wt2 = uwpool.tile([P, KO, H], f16, tag="uw")  # wT reuses slot
                mm_layer(v, wf_sb[1], None, clamp_into(wt2))

                orow = [None]

                def final_writer(m, nt, ps, c=c, v=v, orow=orow):
                    yt = ytp.tile([P, NT], f16, tag="yt")
                    nc.vector.tensor_scalar(
                        yt[:],
                        ps,
                        1.0,
                        -1.0,
                        mybir.AluOpType.min,
                        mybir.AluOpType.max,
                    )
                    if nt == 0:
                        orow[0] = outp.tile([P, H], f16, tag="out", name="ot")
                    ot = orow[0]
                    nc.vector.tensor_add(
                        ot[:, nt * NT:(nt + 1) * NT],
                        yt[:],
                        v[:, m, nt * NT:(nt + 1) * NT],
                    )
                    if nt == NN - 1:
                        # one whole-row DMA per m-block on the (idle after
                        # startup) scalar queue: fewer, larger transfers
                        nc.scalar.dma_start(outr[c, :, m, :], ot[:, :])

                mm_layer(wt2, wf_sb[2], zb_sb, final_writer)

                if c + 1 < CLOC:
                    s_cur = s_next

    nc.compile()  # bacc passes: split multi-waits into event semaphores etc.
    return nc


def _prep_host(x, p_mask, Wp, Wp_diag, Wzp, p_lin_w, p_lin_b, z_lin_w,
               z_lin_b):
    x = np.ascontiguousarray(
        np.asarray(x, dtype=np.float32).reshape(C, H, H).astype(np.float16)
    )
    mask = np.clip(np.asarray(p_mask, dtype=np.float32), -1.0, 1.0)
    mask = np.ascontiguousarray(mask.astype(np.float16))

    Wp = np.asarray(Wp, dtype=np.float32)
    Wp_eff = np.tril(Wp)
    idx = np.arange(H)
    Wp_eff[idx, idx] = np.clip(np.diagonal(Wp), 0.0, 1.0) + np.asarray(
        Wp_diag, dtype=np.float32
    )
    w0 = np.ascontiguousarray(Wp_eff.T.astype(np.float16))
    wf = [
        np.ascontiguousarray(np.asarray(p_lin_w, dtype=np.float32).T.astype(np.float16)),
        np.ascontiguousarray(np.asarray(Wzp, dtype=np.float32).T.astype(np.float16)),
        np.ascontiguousarray(np.asarray(z_lin_w, dtype=np.float32).T.astype(np.float16)),
    ]
    pb = np.asarray(p_lin_b, dtype=np.float32).reshape(1, H).astype(np.float16)
    zb = np.asarray(z_lin_b, dtype=np.float32).reshape(1, H).astype(np.float16)
    return x, mask, w0, wf, pb, zb


def kernel(x, p_mask, Wp, Wp_diag, Wzp, p_lin_w, p_lin_b, z_lin_w, z_lin_b):
    global last_results
    x, mask, w0, wf, pb, zb = _prep_host(
        x, p_mask, Wp, Wp_diag, Wzp, p_lin_w, p_lin_b, z_lin_w, z_lin_b
    )
    has_pb = bool(np.any(pb))
    has_zb = bool(np.any(zb))

    key = (has_pb, has_zb)
    if key not in _cache:
        _cache[key] = _build(has_pb, has_zb)
    nc = _cache[key]

    in_maps = []
    for core in range(NCORES):
        m = {
            "x": x[core * CLOC:(core + 1) * CLOC],
            "mask": mask,
            "w0": w0,
            "w1": wf[0],
            "w2": wf[1],
            "w3": wf[2],
        }
        if has_pb:
            m["pb"] = pb
        if has_zb:
            m["zb"] = zb
        in_maps.append(m)

    want_trace = bool(os.environ.get("BASS_TRACE"))
    try:
        res = run_bass_kernel_spmd(
            nc, in_maps, list(range(NCORES)), trace=want_trace
        )
    except ModuleNotFoundError:
        if not want_trace:
            raise
        # profiling hook unavailable in this environment -- run untraced
        res = run_bass_kernel_spmd(
            nc, in_maps, list(range(NCORES)), trace=False
        )
    last_results = res
    out = np.concatenate([r["out"] for r in res.results], axis=0)
    return out.reshape(1, C, H, H).astype(np.float32)
